# revision 32
# baseline (speedup 1.0000x reference)
"""Sharded Bass kernel for nn_BDRRAA (sparse_attention category).

Sharding (per the hint): the pairwise (S_i x S_j) block is sharded over
sample_i rows across 8 NeuronCores (375 rows -> 3 tiles of 128); the edge
(link) term is data-parallel over the edge list (62500 edges per core).
Each core writes [128, 8] partial sums; the host reduces them (all-reduce
equivalent) and returns links - mat.

Device math per core (measured ~34.8us on HW; framework floor ~13.8us):
  Pairwise, 7 column-chunks (tile 0: 512+1248+1248 so the first sqrt
  fires early; tiles 1-2: 1504+1504), J padded 3000 -> 3008:
    PSUM   = lhsT^T @ rhs          # bf16 lhsT = [-2*Mi; 1; 1; 1], rhs = [Mj; vh; vm; vl]
    t      = sqrt(PSUM + ui)       # ACT, per-partition bias ui = |Mi+eps|^2 + margin
    u      = gb - t                # DVE tensor_sub, bf16 2x_1P mode
    _      = exp(u + beta_i)       # ACT, one [128,3008] exp per tile,
                                   # accum_out -> mat partial column
  (All 7 sqrts are emitted before any exp so the ACT table set switches
   exactly once: sqrt_and_others -> exp_and_others; a dependency-free
   warm-up sqrt hoists the first table load into the preamble window.)
  Edge term via TensorE block-diag trick, 31 chunks of 16 groups x 128 edges:
    PSUM[128,128] += A_chunk^T @ B_chunk   # A = 2*Pi[ei], B = Pj[ej], packed
                                           # 16 groups of 8 dims per chunk
    diag partial = sum_free(PSUM * I)      # DVE stt with accum_out
    s partial    = sum_free(s_pack)        # DVE reduce; s_e = fi[ei] + gj[ej]
  links = sum(diag) + sum(s);  sum_e (a_e . b_e) over the block diagonals
  equals sum_e 2 Pi[ei].Pj[ej].
  Inputs ship as 4 merged HWDGE DMAs ordered by need (pw, hdr, gbb,
  edata); hdr carries f32 sections bitcast through a bf16 tensor.

Identity: sum_d (x_d - y_d + eps)^2 == sum_d (x_d+eps)^2 + (y_d-eps)^2
  - 2 x.y + 8 eps^2.  With ui/vj computed from the *bf16-rounded* points,
  PSUM + ui = |Mi_bf - Mj_bf + eps|^2 + eps^2 + margin > 0 always, so the
  sqrt can't see a negative. vj is split into three bf16 rows (vh+vm+vl)
  so its rounding error (~3e-7) stays far below the margin (1e-4).

kernel.py is self-contained: shapes/sharding hardcoded from the spec.
"""

import numpy as np

N_I, N_J = 50000, 50000
K = 8
D = 8
S_I, S_J = 3000, 3000
N_EDGES = 500000
EPS = np.float32(1e-06)
MARGIN = np.float32(1e-04)
N_CORES = 8

ROWS_PER_CORE = S_I // N_CORES          # 375
N_TILES = 3                             # 3 x 128 = 384 row slots per core
J_PAD = 3008                            # S_J=3000 + 8 pad columns
N_CHUNKS = 7                            # sqrt chunks (tile 0: 3, tiles 1-2: 2)
EDGES_PER_CORE = N_EDGES // N_CORES     # 62500
EDGE_MM = 31                            # 31 chunks x 16 groups x 128 edges
EDGE_SLOTS = EDGE_MM * 16 * 128         # 63488
S_COLS = 492                            # 128*492 = 62976 >= 62500

LAST_HW_EXEC_NS = None
_PROG = None


def _softmax0(x):
    m = x.max(axis=0, keepdims=True)
    e = np.exp(x - m)
    return e / e.sum(axis=0, keepdims=True)


def _prep(beta, gamma, A, Z_i, Z_j, G):
    Zi = _softmax0(Z_i.astype(np.float32))
    Zj = _softmax0(Z_j.astype(np.float32))
    Z = np.concatenate([Zi, Zj], axis=1)
    Gs = 1.0 / (1.0 + np.exp(-G.astype(np.float32)))
    ZG = Z.T * Gs
    colsum = ZG.sum(axis=0)
    M1 = Z @ ZG
    AZC = (A.astype(np.float32) @ (M1 / colsum[None, :])).T
    Pi = (AZC @ Zi).T.astype(np.float32)     # [N_i, d]
    Pj = (AZC @ Zj).T.astype(np.float32)     # [N_j, d]
    return Pi, Pj


def _build_program():
    import concourse.bacc as bacc
    import concourse.mybir as mybir
    import concourse.tile as tile
    from contextlib import ExitStack

    f32 = mybir.dt.float32
    bf16 = mybir.dt.bfloat16

    # Bacc (not plain Bass): its finalize() runs generate_event_semaphores,
    # which splits multi-sem waits into EventSemaphore prefixes — this
    # walrus accepts at most ONE sync wait per compute instruction.
    nc = bacc.Bacc()
    # pw: [11, J_PAD+384] = rhs (cols 0:J_PAD = [Mj; vh; vm; vl]) ++
    # per-tile lhsT (cols J_PAD+128t:J_PAD+128(t+1) = [-2Mi; 1; 1; 1]).
    pw = nc.declare_dram_parameter("pw", [11, J_PAD + N_TILES * 128], bf16, isOutput=False)
    # hdr: raw bytes as bf16 — cols 0:12 = uibeta f32 (bitcast), 12:140 =
    # diag mask bf16, 140:1124 = s_pack f32 (bitcast).
    hdr = nc.declare_dram_parameter("hdr", [128, 12 + 128 + 2 * S_COLS], bf16, isOutput=False)
    gbb = nc.declare_dram_parameter("gbb", [128, J_PAD], bf16, isOutput=False)
    edata = nc.declare_dram_parameter("edata", [128, EDGE_MM, 2, 128], bf16, isOutput=False)
    outp = nc.declare_dram_parameter("out", [128, 8], f32, isOutput=True)

    with tile.TileContext(nc) as tc, ExitStack() as ctx:
        const = ctx.enter_context(tc.tile_pool(name="const", bufs=1))
        tbp = ctx.enter_context(tc.tile_pool(name="tbp", bufs=N_CHUNKS))
        work = ctx.enter_context(tc.tile_pool(name="work", bufs=2))
        acc = ctx.enter_context(tc.tile_pool(name="acc", bufs=1))
        ps = ctx.enter_context(tc.tile_pool(name="ps", bufs=2, space="PSUM"))
        ps0 = ctx.enter_context(tc.tile_pool(name="ps0", bufs=1, space="PSUM"))
        pse = ctx.enter_context(tc.tile_pool(name="pse", bufs=1, space="PSUM"))

        # --- ACT warm-up: a dependency-free sqrt at the top of the ACT
        # stream makes bacc place the sqrt table load in the idle preamble
        # window instead of right before the first real sqrt.
        warm_in = const.tile([1, 2], f32, tag="warm_in")
        nc.vector.memset(warm_in[:], 0.0)
        warm_out = const.tile([1, 2], f32, tag="warm_out")
        nc.scalar.activation(
            warm_out[:], warm_in[:], mybir.ActivationFunctionType.Sqrt,
            bias=warm_in[0:1, 0:1])


        # --- input DMAs, one HWDGE ring, ordered by when data is needed ---
        pw_sb = const.tile([11, J_PAD + N_TILES * 128], bf16, tag="pw")
        nc.sync.dma_start(out=pw_sb[:], in_=pw[:, :])
        hdr_sb = const.tile([128, 12 + 128 + 2 * S_COLS], bf16, tag="hdr")
        nc.sync.dma_start(out=hdr_sb[:], in_=hdr[:, :])
        gb_sb = const.tile([128, J_PAD], bf16, tag="gb")
        nc.sync.dma_start(out=gb_sb[:], in_=gbb[:, :])
        e_sb = const.tile([128, EDGE_MM, 2, 128], bf16, tag="edata")
        nc.sync.dma_start(out=e_sb[:], in_=edata[:, :])

        rhs_sb = pw_sb[:, 0:J_PAD]
        ub_sb = hdr_sb[:, 0:12].bitcast(f32)          # [128, 6] f32
        mask_sb = hdr_sb[:, 12:140]                   # [128, 128] bf16
        s_sb = hdr_sb[:, 140:140 + 2 * S_COLS].bitcast(f32)  # [128, 492] f32

        oc = acc.tile([128, 8], f32, tag="oc")

        # --- pairwise phase 1: matmul + sqrt (sqrt_and_others set) ---
        # Tile 0's first chunk is small (512 cols) so the first sqrt fires
        # ~0.7us earlier; the ACT chain stays saturated from then on.
        tbs = []
        for t in range(N_TILES):
            bounds = [0, 512, 1760, J_PAD] if t == 0 else [0, 1504, J_PAD]
            for h in range(len(bounds) - 1):
                c0, c1 = bounds[h], bounds[h + 1]
                if c1 - c0 <= 512:
                    pt = ps0.tile([128, c1 - c0], f32, tag="pt0")
                else:
                    pt = ps.tile([128, c1 - c0], f32, tag="pt")
                for i in range(0, c1 - c0, 512):
                    w = min(512, c1 - c0 - i)
                    nc.tensor.matmul(
                        out=pt[:, i:i + w],
                        lhsT=pw_sb[:, J_PAD + 128 * t:J_PAD + 128 * (t + 1)],
                        rhs=rhs_sb[:, c0 + i:c0 + i + w],
                        start=True, stop=True,
                    )
                tb = tbp.tile([128, c1 - c0], bf16, tag="tb")
                nc.scalar.activation(
                    tb[:], pt[:], mybir.ActivationFunctionType.Sqrt,
                    bias=ub_sb[:, 2 * t:2 * t + 1], scale=1.0,
                )
                tbs.append((t, c0, c1, tb))

        # --- edge matmuls (PE, accumulate into one PSUM [128,128]) ---
        pe = pse.tile([128, 128], f32, tag="pe")
        for c in range(EDGE_MM):
            nc.tensor.matmul(
                out=pe[:],
                lhsT=e_sb[:, c, 0, :], rhs=e_sb[:, c, 1, :],
                start=(c == 0), stop=(c == EDGE_MM - 1),
            )

        # --- pairwise phase 2: u = gb - t (DVE, per half-tile, bf16 2x),
        # then ONE exp+accum per tile (FD=3072 amortizes ACT overhead) ---
        us = []
        for t in range(N_TILES):
            u = work.tile([128, J_PAD], bf16, tag="u", bufs=3)
            us.append(u)
        for k, (t, c0, c1, tb) in enumerate(tbs):
            # tensor_tensor subtract (not stt): bf16 TT has a 2x_1P uop,
            # InstTensorScalarPtr runs 1x only (measured 1747ns vs 957ns).
            nc.vector.tensor_sub(
                us[t][:, c0:c1], gb_sb[:, c0:c1], tb[:],
            )
            if c1 == J_PAD:
                eb = work.tile([128, J_PAD], bf16, tag="eb")
                nc.scalar.activation(
                    eb[:], us[t][:], mybir.ActivationFunctionType.Exp,
                    bias=ub_sb[:, 2 * t + 1:2 * t + 2], scale=1.0,
                    accum_out=oc[:, t:t + 1],
                )

        # --- edge extract: block-diagonal of PSUM + s reduction ---
        dg = work.tile([128, 128], f32, tag="dg")
        nc.vector.scalar_tensor_tensor(
            dg[:], in0=pe[:], scalar=1.0, in1=mask_sb,
            op0=mybir.AluOpType.mult, op1=mybir.AluOpType.mult,
            accum_out=oc[:, 3:4],
        )
        nc.vector.tensor_reduce(
            oc[:, 4:5], s_sb, axis=mybir.AxisListType.X,
            op=mybir.AluOpType.add,
        )

        nc.sync.dma_start(out=outp[:, :], in_=oc[:])
    nc.finalize()
    return nc


def _host_partials(beta, gamma, Pi, Pj, si, sj, ssi, ssj):
    """Build per-core device inputs. Returns in_maps list."""
    import ml_dtypes

    bf = ml_dtypes.bfloat16

    def to_bf(x):
        return x.astype(bf)

    Mi = to_bf(Pi[si]).astype(np.float32)     # bf16-rounded points, in f32
    Mj = to_bf(Pj[sj]).astype(np.float32)
    bs = beta[si].astype(np.float32)
    gs = gamma[sj].astype(np.float32)

    ui_full = ((Mi + EPS) ** 2).sum(1).astype(np.float32) + MARGIN
    vj = ((Mj - EPS) ** 2).sum(1).astype(np.float32)
    vh = vj.astype(bf)
    vm = (vj - vh.astype(np.float32)).astype(bf)
    vl = (vj - vh.astype(np.float32) - vm.astype(np.float32)).astype(bf)

    rhsj = np.zeros((11, J_PAD), dtype=bf)
    rhsj[:8, :S_J] = to_bf(Mj.T)
    rhsj[8, :S_J] = vh
    rhsj[9, :S_J] = vm
    rhsj[10, :S_J] = vl

    gbrow = np.full((J_PAD,), -1e9, dtype=np.float32)
    gbrow[:S_J] = gs
    gbb = np.ascontiguousarray(
        np.broadcast_to(gbrow.astype(bf), (128, J_PAD)))

    mask_u16 = np.eye(128, dtype=bf).view(np.uint16)

    # edge tables (f32 fi/gj; cross term in bf16 on the PE)
    sqPi = (Pi ** 2).sum(1)
    sPi = Pi.sum(1)
    sqPj = (Pj ** 2).sum(1)
    sPj = Pj.sum(1)
    fi = (beta - sqPi - 2 * EPS * sPi).astype(np.float32)
    gj = (gamma - sqPj + 2 * EPS * sPj - 8 * EPS * EPS).astype(np.float32)
    Pi_bf = to_bf(2.0 * Pi)                   # [N_i, 8] bf16
    Pj_bf = to_bf(Pj)

    in_maps = []
    for c in range(N_CORES):
        r0 = c * ROWS_PER_CORE
        rows = slice(r0, r0 + ROWS_PER_CORE)
        lhst = np.zeros((11, N_TILES, 128), dtype=bf)
        uibeta = np.zeros((128, N_TILES, 2), dtype=np.float32)
        uibeta[:, :, 0] = 1.0
        uibeta[:, :, 1] = -1e9
        MiT = Mi[rows].T                       # [8, 375]
        for t in range(N_TILES):
            n0 = t * 128
            n1 = min(n0 + 128, ROWS_PER_CORE)
            w = n1 - n0
            if w <= 0:
                continue
            lhst[:8, t, :w] = to_bf(-2.0 * MiT[:, n0:n1])
            lhst[8, t, :w] = 1.0
            lhst[9, t, :w] = 1.0
            lhst[10, t, :w] = 1.0
            uibeta[:w, t, 0] = ui_full[r0 + n0:r0 + n1]
            uibeta[:w, t, 1] = bs[r0 + n0:r0 + n1]

        e0 = c * EDGES_PER_CORE
        ei = ssi[e0:e0 + EDGES_PER_CORE]
        ej = ssj[e0:e0 + EDGES_PER_CORE]
        ne = EDGES_PER_CORE
        A = np.zeros((EDGE_SLOTS, 8), dtype=bf)
        B = np.zeros((EDGE_SLOTS, 8), dtype=bf)
        A[:ne] = Pi_bf[ei]
        B[:ne] = Pj_bf[ej]
        # [chunk, group, edge, dim] -> edata[p, chunk, {A,B}, group*8+dim]
        A4 = A.reshape(EDGE_MM, 16, 128, 8).transpose(2, 0, 1, 3).reshape(
            128, EDGE_MM, 1, 128)
        B4 = B.reshape(EDGE_MM, 16, 128, 8).transpose(2, 0, 1, 3).reshape(
            128, EDGE_MM, 1, 128)
        edata = np.concatenate([A4, B4], axis=2)  # [128, 31, 2, 128]

        s_e = fi[ei] + gj[ej]
        spk = np.zeros((128 * S_COLS,), dtype=np.float32)
        spk[:ne] = s_e
        spk = spk.reshape(128, S_COLS)

        # pw = rhs ++ per-tile lhsT, both partition-dim 11
        pw = np.zeros((11, J_PAD + N_TILES * 128), dtype=bf)
        pw[:, :J_PAD] = rhsj
        pw[:, J_PAD:] = lhst.reshape(11, N_TILES * 128)

        # hdr = raw bytes: uibeta f32 | mask bf16 | spk f32
        hdr = np.concatenate([
            uibeta.reshape(128, 6).view(np.uint16),   # 12 cols
            mask_u16,                                  # 128 cols
            spk.view(np.uint16),                       # 984 cols
        ], axis=1).view(bf)

        in_maps.append({
            "pw": pw, "hdr": np.ascontiguousarray(hdr), "gbb": gbb,
            "edata": np.ascontiguousarray(edata),
        })
    return in_maps


def _host_fallback(beta, gamma, Pi, Pj, si, sj, ssi, ssj):
    """Host compute mirroring the device sharding: 8 thread-parallel row
    blocks for the pairwise term, 8 edge chunks for the link term."""
    from concurrent.futures import ThreadPoolExecutor

    Mi = Pi[si]
    Mj = Pj[sj]
    bs = beta[si]
    gs = gamma[sj]
    ui = ((Mi + EPS) ** 2).sum(1)
    vj = ((Mj - EPS) ** 2).sum(1)
    MjT = np.ascontiguousarray(Mj.T)

    def pair_block(c):
        r = slice(c * ROWS_PER_CORE, (c + 1) * ROWS_PER_CORE)
        d2 = Mi[r] @ MjT
        d2 *= -2.0
        d2 += ui[r][:, None]
        d2 += vj[None, :]
        np.maximum(d2, 0.0, out=d2)
        np.sqrt(d2, out=d2)
        d2 -= bs[r][:, None]
        d2 -= gs[None, :]
        d2 *= -1.0
        np.exp(d2, out=d2)
        return d2.sum(dtype=np.float64)

    sqPi = (Pi ** 2).sum(1); sPi = Pi.sum(1)
    sqPj = (Pj ** 2).sum(1); sPj = Pj.sum(1)
    fi = beta - sqPi - 2 * EPS * sPi
    gj = gamma - sqPj + 2 * EPS * sPj - 8 * EPS * EPS

    def edge_block(c):
        e = slice(c * EDGES_PER_CORE, (c + 1) * EDGES_PER_CORE)
        ei = ssi[e]; ej = ssj[e]
        cross = np.einsum('ed,ed->e', Pi[ei], Pj[ej])
        return (fi[ei].sum(dtype=np.float64) + gj[ej].sum(dtype=np.float64)
                + 2.0 * cross.sum(dtype=np.float64))

    with ThreadPoolExecutor(max_workers=N_CORES) as ex:
        mats = list(ex.map(pair_block, range(N_CORES)))
        links = list(ex.map(edge_block, range(N_CORES)))
    return np.float32(float(sum(links)) - float(sum(mats)))


def kernel(beta, gamma, A, Z_i, Z_j, G, sample_i_idx, sample_j_idx,
           sparse_sample_i, sparse_sample_j):
    global LAST_HW_EXEC_NS, _PROG
    beta = np.asarray(beta, dtype=np.float32)
    gamma = np.asarray(gamma, dtype=np.float32)
    si = np.asarray(sample_i_idx).astype(np.int64)
    sj = np.asarray(sample_j_idx).astype(np.int64)
    ssi = np.asarray(sparse_sample_i).astype(np.int64)
    ssj = np.asarray(sparse_sample_j).astype(np.int64)

    Pi, Pj = _prep(beta, gamma, np.asarray(A), np.asarray(Z_i),
                   np.asarray(Z_j), np.asarray(G))

    try:
        import os
        if os.environ.get("BDRRAA_HOST"):
            raise RuntimeError("host path forced (BDRRAA_HOST set)")
        from concourse.bass_utils import run_bass_kernel_spmd

        in_maps = _host_partials(beta, gamma, Pi, Pj, si, sj, ssi, ssj)
        if _PROG is None:
            _PROG = _build_program()
        trace = bool(os.environ.get("BASS_TRACE"))
        kw = {}
        if trace and os.environ.get("BASS_TRACE_ALL_CORES"):
            kw["trace_cores"] = list(range(N_CORES))
        res = run_bass_kernel_spmd(
            _PROG, in_maps, core_ids=list(range(N_CORES)), trace=trace, **kw,
        )
        LAST_HW_EXEC_NS = getattr(res, "exec_time_ns", None)
        outs = [np.asarray(r["out"], dtype=np.float64) for r in res.results]
        mat = float(sum(o[:, 0:3].sum() for o in outs))
        links = float(sum(o[:, 3:5].sum() for o in outs))
        return np.float32(links - mat)
    except Exception as e:  # pragma: no cover - device-unavailable fallback
        if str(e) != "host path forced (BDRRAA_HOST set)":
            print(f"[kernel] device path failed ({type(e).__name__}: {e}); "
                  f"falling back to host compute")
        return _host_fallback(beta, gamma, Pi, Pj, si, sj, ssi, ssj)
